# revision 1
# baseline (speedup 1.0000x reference)
"""3x3 same-padding conv (C_in=256, H=W=512, C_out=256) + bias on 8 trn2 cores.

Sharding: H split across 8 cores (64 output rows each, 1-row halo included in
each core's input slice on the host — no device-side halo exchange needed).

Per core the conv is computed as accumulated PE matmuls in float32r
(TF32-like, ~1e-4 rel err, full PE rate at N=512):
  out[co, y, :] = sum_{kh,kw,ci_half} W[kh,kw,ci_half,co].T @ xpad[ci_half, y+kh, kw:kw+512]
18 matmuls (3 kh x 2 ci_half x 3 kw) accumulate into one PSUM bank per
(row, co_half); ScalarE adds bias while draining PSUM -> SBUF; DMA out.

The first/last row blocks are small so the first matmul only waits on a
small input transfer (weights are split per co_half for the same reason)
and the final store is short.
"""
import numpy as np

import concourse.bacc as bacc
import concourse.mybir as mybir
import concourse.tile as tile
from concourse import bass_utils

NCORES = 8
CIN = 256
COUT = 256
H = 512
W = 512
RPC = H // NCORES          # output rows per core (64)
WPAD = W + 2               # width incl. zero pad cols
NTAPS = 36                 # 3*3 * 2 ci halves * 2 co halves weight tiles
BLOCKS = [2, 6] + [8] * 6 + [6, 2]   # row-block sizes (sum = RPC)
assert sum(BLOCKS) == RPC

_CACHED_NC = {}


def _build_nc(repeat=1, in_dt="float32r"):
    f32 = mybir.dt.float32
    f32r = getattr(mybir.dt, in_dt)
    nc = bacc.Bacc("TRN2", target_bir_lowering=False, debug=False,
                   num_devices=NCORES)

    xs_d = nc.dram_tensor("xs", [CIN, RPC + 2, WPAD], f32r, kind="ExternalInput")
    # weight layout: [ci_half partition, bo*18 + (kh*3+kw)*2 + bi, co]
    wt_d = nc.dram_tensor("wt", [128, NTAPS, 128], f32r, kind="ExternalInput")
    bias_d = nc.dram_tensor("bias", [128, 2], f32, kind="ExternalInput")
    out_d = nc.dram_tensor("out", [128, 2, RPC, W], f32, kind="ExternalOutput")
    # tiny output: fetching it forces execution completion without a bulk D2H
    done_d = nc.dram_tensor("done", [1, 1], f32, kind="ExternalOutput")

    mxb = max(BLOCKS)
    with tile.TileContext(nc) as tc:
        with (
            tc.tile_pool(name="const", bufs=1) as cpool,
            tc.tile_pool(name="xin", bufs=3) as xpool,
            tc.tile_pool(name="oout", bufs=2) as opool,
            tc.tile_pool(name="psum", bufs=8, space="PSUM") as psum,
        ):
            # x block 0 first, then weights split over 3 queues so the
            # first accumulation group's taps arrive earliest
            b0 = BLOCKS[0] + 2
            xa0 = xpool.tile([128, mxb + 2, WPAD], f32r, tag="xa")
            nc.scalar.dma_start(xa0[:, 0:b0, :], xs_d[0:128, 0:b0, :])
            xb0 = xpool.tile([128, mxb + 2, WPAD], f32r, tag="xb")
            nc.scalar.dma_start(xb0[:, 0:b0, :], xs_d[128:256, 0:b0, :])
            wtA_s = cpool.tile([128, 9, 128], f32r, tag="wtA")
            nc.sync.dma_start(wtA_s[:], wt_d[:, 0:9, :])
            wtB_s = cpool.tile([128, 9, 128], f32r, tag="wtB")
            nc.sync.dma_start(wtB_s[:], wt_d[:, 9:18, :])
            wt1_s = cpool.tile([128, 18, 128], f32r, tag="wt1")
            nc.sync.dma_start(wt1_s[:], wt_d[:, 18:36, :])
            bias_s = cpool.tile([128, 2], f32, tag="bias")
            nc.sync.dma_start(bias_s[:], bias_d[:])
            nc.sync.dma_start(done_d[:], bias_d[0:1, 0:1])

            def wtap(bo, j):
                if bo == 1:
                    return wt1_s[:, j, :]
                return wtA_s[:, j, :] if j < 9 else wtB_s[:, j - 9, :]

            for rep in range(repeat):
                r0 = 0
                for blk_i, rblk in enumerate(BLOCKS):
                    if rep == 0 and blk_i == 0:
                        xa, xb = xa0, xb0
                    else:
                        xa = xpool.tile([128, mxb + 2, WPAD], f32r, tag="xa")
                        nc.sync.dma_start(xa[:, 0:rblk + 2, :],
                                          xs_d[0:128, r0:r0 + rblk + 2, :])
                        xb = xpool.tile([128, mxb + 2, WPAD], f32r, tag="xb")
                        nc.sync.dma_start(xb[:, 0:rblk + 2, :],
                                          xs_d[128:256, r0:r0 + rblk + 2, :])
                    oa = opool.tile([128, mxb, W], f32, tag="oa")
                    ob = opool.tile([128, mxb, W], f32, tag="ob")
                    for yy in range(rblk):
                        for bo in range(2):
                            acc = psum.tile([128, W], f32, tag="acc")
                            k = 0
                            for kh in range(3):
                                for bi in range(2):
                                    xt = xa if bi == 0 else xb
                                    for kw in range(3):
                                        j = (kh * 3 + kw) * 2 + bi
                                        nc.tensor.matmul(
                                            acc[:],
                                            wtap(bo, j),
                                            xt[:, yy + kh, kw:kw + W],
                                            start=(k == 0),
                                            stop=(k == NTAPS // 2 - 1),
                                        )
                                        k += 1
                            ot = oa if bo == 0 else ob
                            nc.scalar.activation(
                                ot[:, yy, :], acc[:],
                                mybir.ActivationFunctionType.Identity,
                                bias=bias_s[:, bo:bo + 1],
                            )
                    nc.sync.dma_start(out_d[:, 0, r0:r0 + rblk, :],
                                      oa[:, 0:rblk, :])
                    nc.sync.dma_start(out_d[:, 1, r0:r0 + rblk, :],
                                      ob[:, 0:rblk, :])
                    r0 += rblk

    nc.compile()
    return nc


def _get_nc(repeat=1, in_dt="float32r"):
    key = (repeat, in_dt)
    if key not in _CACHED_NC:
        _CACHED_NC[key] = _build_nc(repeat, in_dt)
    return _CACHED_NC[key]


def _prep_inputs(x, W_, b, in_dt="float32r"):
    npdt = np.float32
    if in_dt == "bfloat16":
        import ml_dtypes
        npdt = ml_dtypes.bfloat16
    xs_all = np.zeros((NCORES, CIN, RPC + 2, WPAD), npdt)
    for m in range(NCORES):
        g0 = max(0, m * RPC - 1)
        g1 = min(H, m * RPC + RPC + 1)
        r0 = g0 - (m * RPC - 1)
        xs_all[m, :, r0:r0 + (g1 - g0), 1:1 + W] = x[:, g0:g1, :]
    # [kh, kw, ci, co] -> [ci_p, bo, kh, kw, bi, co_m] -> [128, 36, 128]
    wt = np.ascontiguousarray(
        W_.reshape(3, 3, 2, 128, 2, 128).transpose(3, 4, 0, 1, 2, 5)
        .reshape(128, NTAPS, 128).astype(npdt))
    bias = np.ascontiguousarray(b.reshape(2, 128).T)
    return xs_all, wt, bias


def kernel(x, W, b, _trace=False):
    x = np.asarray(x, dtype=np.float32)
    W = np.asarray(W, dtype=np.float32)
    b = np.asarray(b, dtype=np.float32)
    nc = _get_nc()
    xs_all, wt, bias = _prep_inputs(x, W, b)
    in_maps = [{"xs": xs_all[m], "wt": wt, "bias": bias} for m in range(NCORES)]
    res = bass_utils.run_bass_kernel_spmd(
        nc, in_maps, list(range(NCORES)), trace=_trace)
    arr = np.stack([res.results[m]["out"] for m in range(NCORES)], axis=0)
    # [m, p, bo, yy, x] -> [bo, p, m, yy, x] -> [C_out, H, W]
    full = arr.transpose(2, 1, 0, 3, 4).reshape(COUT, H, 512)
    if _trace:
        return full, res
    return full



# revision 6
# speedup vs baseline: 1.4161x; 1.4161x over previous
"""3x3 same-padding conv (C_in=256, H=W=512, C_out=256) + bias on 8 trn2 cores.

Sharding: H split across 8 cores (64 output rows each, 1-row halo included in
each core's input slice on the host — no device-side halo exchange needed).

Per core: Winograd F(2,3) along H only. Each tile-row ty produces output rows
(2ty, 2ty+1) from padded input rows d0..d3 = xpad[2ty..2ty+3]:
  V0 = d0 - d2, V1 = d1 + d2, V2 = d2 - d1, V3 = d1 - d3   (GpSimd)
  M[i] = sum_{kw, ci_half} U[i,kw].T @ V[i][:, kw:kw+512]  (PE, 6 mm per bank)
  Y0 = M0 + M1 + M2 + b ; Y1 = M1 - M2 - M3 + b            (VectorE from PSUM)
with U[i,kw] = sum_kh G[i,kh] W[kh,kw] precomputed on host (G = F(2,3) weight
transform). 24 N=512 matmuls per 2 output rows per co_half instead of 36 for
direct conv: 1536 total vs 2304 (1.5x fewer PE cycles). fp32r (TF32-like)
keeps ~1e-4 rel err. In-transform runs on GpSimd and the inverse on VectorE
so the PE never waits on either.
"""
import numpy as np

import concourse.bacc as bacc
import concourse.mybir as mybir
import concourse.tile as tile
from concourse import bass_utils

NCORES = 8
CIN = 256
COUT = 256
H = 512
W = 512
RPC = H // NCORES          # output rows per core (64)
NTY = RPC // 2             # winograd tile-rows per core (32)
WPAD = W + 2               # width incl. zero pad cols
NTAPS = 48                 # 2 co_half * 4 i * 3 kw * 2 ci_half weight tiles
TBLOCKS = [1, 3, 6, 6, 6, 6, 4]   # tile-row block sizes (sum = NTY)
assert sum(TBLOCKS) == NTY

_CACHED_NC = {}


def _build_nc():
    f32 = mybir.dt.float32
    f32r = mybir.dt.float32r
    nc = bacc.Bacc("TRN2", target_bir_lowering=False, debug=False,
                   num_devices=NCORES)

    xs_d = nc.dram_tensor("xs", [CIN, RPC + 2, WPAD], f32, kind="ExternalInput")
    # weight layout: [ci_lo, bo*24 + (i*3+kw)*2 + bi, co_lo]
    wt_d = nc.dram_tensor("wt", [128, NTAPS, 128], f32r, kind="ExternalInput")
    bias_d = nc.dram_tensor("bias", [128, 2], f32, kind="ExternalInput")
    out_d = nc.dram_tensor("out", [128, 2, RPC, W], f32, kind="ExternalOutput")
    # tiny output: fetching it forces execution completion without a bulk D2H
    done_d = nc.dram_tensor("done", [1, 1], f32, kind="ExternalOutput")

    mxb = max(TBLOCKS)
    xrows = 2 * mxb + 2
    with tile.TileContext(nc) as tc:
        with (
            tc.tile_pool(name="const", bufs=1) as cpool,
            tc.tile_pool(name="xin", bufs=2) as xpool,
            tc.tile_pool(name="vbuf", bufs=2) as vpool,
            tc.tile_pool(name="tbuf", bufs=4) as tpool,
            tc.tile_pool(name="oout", bufs=4) as opool,
            tc.tile_pool(name="psum", bufs=8, space="PSUM") as psum,
        ):
            # x block 0 first, then weights split over queues so the first
            # accumulation group's taps arrive earliest
            b0 = 2 * TBLOCKS[0] + 2
            xa0 = xpool.tile([128, xrows, WPAD], f32, tag="xa")
            nc.scalar.dma_start(xa0[:, 0:b0, :], xs_d[0:128, 0:b0, :])
            xb0 = xpool.tile([128, xrows, WPAD], f32, tag="xb")
            nc.scalar.dma_start(xb0[:, 0:b0, :], xs_d[128:256, 0:b0, :])
            wtA_s = cpool.tile([128, 12, 128], f32r, tag="wtA")
            nc.sync.dma_start(wtA_s[:], wt_d[:, 0:12, :])
            wtB_s = cpool.tile([128, 12, 128], f32r, tag="wtB")
            nc.sync.dma_start(wtB_s[:], wt_d[:, 12:24, :])
            wt1_s = cpool.tile([128, 24, 128], f32r, tag="wt1")
            nc.sync.dma_start(wt1_s[:], wt_d[:, 24:48, :])
            bias_s = cpool.tile([128, 2], f32, tag="bias")
            nc.sync.dma_start(bias_s[:], bias_d[:])
            nc.sync.dma_start(done_d[:], bias_d[0:1, 0:1])

            def wtap(bo, i, kw, bi):
                j = bo * 24 + (i * 3 + kw) * 2 + bi
                if j < 12:
                    return wtA_s[:, j, :]
                if j < 24:
                    return wtB_s[:, j - 12, :]
                return wt1_s[:, j - 24, :]

            ty0 = 0
            for blk_i, tblk in enumerate(TBLOCKS):
                nrows = 2 * tblk + 2
                if blk_i == 0:
                    xa, xb = xa0, xb0
                else:
                    r0 = 2 * ty0
                    xa = xpool.tile([128, xrows, WPAD], f32, tag="xa")
                    nc.sync.dma_start(xa[:, 0:nrows, :],
                                      xs_d[0:128, r0:r0 + nrows, :])
                    xb = xpool.tile([128, xrows, WPAD], f32, tag="xb")
                    nc.sync.dma_start(xb[:, 0:nrows, :],
                                      xs_d[128:256, r0:r0 + nrows, :])
                for lty in range(tblk):
                    ty = ty0 + lty
                    lr = 2 * lty
                    vs = []
                    for xt, vtag in ((xa, "va"), (xb, "vb")):
                        v = vpool.tile([128, 4, WPAD], f32r, tag=vtag)
                        nc.gpsimd.tensor_sub(v[:, 0, :], xt[:, lr + 0, :],
                                             xt[:, lr + 2, :])
                        nc.gpsimd.tensor_add(v[:, 1, :], xt[:, lr + 1, :],
                                             xt[:, lr + 2, :])
                        nc.gpsimd.tensor_sub(v[:, 2, :], xt[:, lr + 2, :],
                                             xt[:, lr + 1, :])
                        nc.gpsimd.tensor_sub(v[:, 3, :], xt[:, lr + 1, :],
                                             xt[:, lr + 3, :])
                        vs.append(v)
                    for bo in range(2):
                        accs = []
                        for i in range(4):
                            acc = psum.tile([128, W], f32, tag="acc")
                            for bi in range(2):
                                for kw in range(3):
                                    nc.tensor.matmul(
                                        acc[:],
                                        wtap(bo, i, kw, bi),
                                        vs[bi][:, i, kw:kw + W],
                                        start=(bi == 0 and kw == 0),
                                        stop=(bi == 1 and kw == 2),
                                    )
                            accs.append(acc)
                        bvec = bias_s[:, bo:bo + 1]
                        ot = opool.tile([128, 2, W], f32, tag="out")
                        # Y0 = (M0 + b) + M1 + M2 ; Y1 = (-M3 + b) + M1 - M2
                        s0 = tpool.tile([128, W], f32, tag="s")
                        nc.scalar.activation(
                            s0[:], accs[0][:],
                            mybir.ActivationFunctionType.Identity, bias=bvec)
                        s1 = tpool.tile([128, W], f32, tag="s")
                        nc.scalar.activation(
                            s1[:], accs[3][:],
                            mybir.ActivationFunctionType.Identity, bias=bvec,
                            scale=-1.0)
                        t0 = tpool.tile([128, W], f32, tag="t")
                        nc.vector.tensor_add(t0[:], s0[:], accs[1][:])
                        nc.vector.tensor_add(ot[:, 0, :], t0[:], accs[2][:])
                        t1 = tpool.tile([128, W], f32, tag="t")
                        nc.vector.tensor_add(t1[:], s1[:], accs[1][:])
                        nc.vector.tensor_sub(ot[:, 1, :], t1[:], accs[2][:])
                        nc.sync.dma_start(out_d[:, bo, 2 * ty:2 * ty + 2, :],
                                          ot[:])
                ty0 += tblk

    nc.compile()
    return nc


def _get_nc():
    if "nc" not in _CACHED_NC:
        _CACHED_NC["nc"] = _build_nc()
    return _CACHED_NC["nc"]


def _prep_inputs(x, W_, b):
    xs_all = np.zeros((NCORES, CIN, RPC + 2, WPAD), np.float32)
    for m in range(NCORES):
        g0 = max(0, m * RPC - 1)
        g1 = min(H, m * RPC + RPC + 1)
        r0 = g0 - (m * RPC - 1)
        xs_all[m, :, r0:r0 + (g1 - g0), 1:1 + W] = x[:, g0:g1, :]
    # winograd F(2,3) weight transform along kh
    G = np.array([[1, 0, 0], [0.5, 0.5, 0.5], [0.5, -0.5, 0.5], [0, 0, 1]],
                 np.float32)
    U = np.einsum('ik,kwab->iwab', G, W_)      # [4, 3, CIN, COUT]
    # [i, kw, ci, co] -> [ci_lo, co_hi, i, kw, ci_hi, co_lo] -> [128, 48, 128]
    wt = np.ascontiguousarray(
        U.reshape(4, 3, 2, 128, 2, 128).transpose(3, 4, 0, 1, 2, 5)
        .reshape(128, NTAPS, 128).astype(np.float32))
    bias = np.ascontiguousarray(b.reshape(2, 128).T)
    return xs_all, wt, bias


def kernel(x, W, b, _trace=False):
    x = np.asarray(x, dtype=np.float32)
    W = np.asarray(W, dtype=np.float32)
    b = np.asarray(b, dtype=np.float32)
    nc = _get_nc()
    xs_all, wt, bias = _prep_inputs(x, W, b)
    in_maps = [{"xs": xs_all[m], "wt": wt, "bias": bias} for m in range(NCORES)]
    res = bass_utils.run_bass_kernel_spmd(
        nc, in_maps, list(range(NCORES)), trace=_trace)
    arr = np.stack([res.results[m]["out"] for m in range(NCORES)], axis=0)
    # [m, p, bo, yy, x] -> [bo, p, m, yy, x] -> [C_out, H, W]
    full = arr.transpose(2, 1, 0, 3, 4).reshape(COUT, H, 512)
    if _trace:
        return full, res
    return full


# revision 10
# speedup vs baseline: 1.5214x; 1.0743x over previous
"""3x3 same-padding conv (C_in=256, H=W=512, C_out=256) + bias on 8 trn2 cores.

Sharding: H split across 8 cores (64 output rows each, 1-row halo included in
each core's input slice on the host — no device-side halo exchange needed).

Per core: Winograd F(2,3) along H only. Each tile-row ty produces output rows
(2ty, 2ty+1) from padded input rows d0..d3 = xpad[2ty..2ty+3]:
  V0 = d0 - d2, V1 = d1 + d2, V2 = d2 - d1, V3 = d1 - d3
  M[i] = sum_{kw, ci_half} U[i,kw].T @ V[i][:, kw:kw+512]  (PE, 6 mm per bank)
  Y0 = M0 + M1 + M2 + b ; Y1 = M1 - M2 - M3 + b
with U[i,kw] = sum_kh G[i,kh] W[kh,kw] precomputed on host (G = F(2,3) weight
transform). 24 N=512 matmuls per 2 output rows per co_half instead of 36 for
direct conv: 1536 total vs 2304 (1.5x fewer PE cycles).

x / V / weights all bf16 (~2e-3 rel err vs the 2e-2 gate): halves DMA and
SBUF, doubles VectorE rate, and bf16 weights get a separate LDWEIGHTS + FWL
so the weight load hides behind the previous matmul. PSUM accum stays fp32.

Engine split: input transform for ci-half A on GpSimd, ci-half B on VectorE
(software-pipelined one tile-row ahead so the PE never waits); inverse on
VectorE reading PSUM banks 1,2 while ScalarE folds the bias into banks 0,3
(s0 = M0 + b, s1 = -M3 + b). x-block DMAs ride the Scalar/Vector queues
(never gated behind output DMAs, which live on the Sync queue); the first
three blocks are issued upfront to fill the pipeline.
"""
import numpy as np

import concourse.bacc as bacc
import concourse.mybir as mybir
import concourse.tile as tile
from concourse import bass_utils

NCORES = 8
CIN = 256
COUT = 256
H = 512
W = 512
RPC = H // NCORES          # output rows per core (64)
NTY = RPC // 2             # winograd tile-rows per core (32)
WPAD = W + 2               # width incl. zero pad cols
NTAPS = 48                 # 2 co_half * 4 i * 3 kw * 2 ci_half weight tiles
TBLOCKS = [1, 2, 3, 6, 6, 6, 4, 4]   # tile-row block sizes (sum = NTY)
assert sum(TBLOCKS) == NTY

_CACHED_NC = {}


def _build_nc():
    f32 = mybir.dt.float32
    bf16 = mybir.dt.bfloat16
    nc = bacc.Bacc("TRN2", target_bir_lowering=False, debug=False,
                   num_devices=NCORES)

    xs_d = nc.dram_tensor("xs", [CIN, RPC + 2, WPAD], bf16, kind="ExternalInput")
    # weight layout: [ci_lo, bo*24 + (i*3+kw)*2 + bi, co_lo]
    wt_d = nc.dram_tensor("wt", [128, NTAPS, 128], bf16, kind="ExternalInput")
    bias_d = nc.dram_tensor("bias", [128, 2], f32, kind="ExternalInput")
    out_d = nc.dram_tensor("out", [128, 2, RPC, W], f32, kind="ExternalOutput")
    # tiny output: fetching it forces execution completion without a bulk D2H
    done_d = nc.dram_tensor("done", [1, 1], f32, kind="ExternalOutput")

    mxb = max(TBLOCKS)
    xrows = 2 * mxb + 2

    # block id and local row for each tile-row
    blk_of = []
    lr_of = []
    ty0s = []
    t = 0
    for bi_, tb in enumerate(TBLOCKS):
        ty0s.append(t)
        for l in range(tb):
            blk_of.append(bi_)
            lr_of.append(2 * l)
        t += tb

    with tile.TileContext(nc) as tc:
        with (
            tc.tile_pool(name="const", bufs=1) as cpool,
            tc.tile_pool(name="xin", bufs=3) as xpool,
            tc.tile_pool(name="vbuf", bufs=2) as vpool,
            tc.tile_pool(name="tbuf", bufs=4) as tpool,
            tc.tile_pool(name="oout", bufs=4) as opool,
            tc.tile_pool(name="psum", bufs=8, space="PSUM") as psum,
        ):
            xtiles = []

            def load_block(bi_):
                r0 = 2 * ty0s[bi_]
                nrows = 2 * TBLOCKS[bi_] + 2
                xa = xpool.tile([128, xrows, WPAD], bf16, tag="xa")
                nc.scalar.dma_start(xa[:, 0:nrows, :],
                                    xs_d[0:128, r0:r0 + nrows, :])
                xb = xpool.tile([128, xrows, WPAD], bf16, tag="xb")
                nc.gpsimd.dma_start(xb[:, 0:nrows, :],
                                    xs_d[128:256, r0:r0 + nrows, :])
                xtiles.append((xa, xb))

            # x blocks 0-2 upfront (pipeline fill); weights split over queues
            # so the first accumulation group's taps arrive earliest
            load_block(0)
            wtA_s = cpool.tile([128, 12, 128], bf16, tag="wtA")
            nc.sync.dma_start(wtA_s[:], wt_d[:, 0:12, :])
            wtB_s = cpool.tile([128, 12, 128], bf16, tag="wtB")
            nc.sync.dma_start(wtB_s[:], wt_d[:, 12:24, :])
            wt1_s = cpool.tile([128, 24, 128], bf16, tag="wt1")
            nc.sync.dma_start(wt1_s[:], wt_d[:, 24:48, :])
            bias_s = cpool.tile([128, 2], f32, tag="bias")
            nc.sync.dma_start(bias_s[:], bias_d[:])
            nc.sync.dma_start(done_d[:], bias_d[0:1, 0:1])
            load_block(1)
            load_block(2)

            def wtap(bo, i, kw, bi):
                j = bo * 24 + (i * 3 + kw) * 2 + bi
                if j < 12:
                    return wtA_s[:, j, :]
                if j < 24:
                    return wtB_s[:, j - 12, :]
                return wt1_s[:, j - 24, :]

            def make_v(ty):
                """Input transform for tile-row ty: ci-half A on GpSimd,
                ci-half B on VectorE. Returns (va, vb)."""
                bi_ = blk_of[ty]
                lr = lr_of[ty]
                xa, xb = xtiles[bi_]
                va = vpool.tile([128, 4, WPAD], bf16, tag="va")
                vb = vpool.tile([128, 4, WPAD], bf16, tag="vb")
                for eng, xt, v in ((nc.gpsimd, xa, va), (nc.vector, xb, vb)):
                    eng.tensor_sub(v[:, 0, :], xt[:, lr + 0, :], xt[:, lr + 2, :])
                    eng.tensor_add(v[:, 1, :], xt[:, lr + 1, :], xt[:, lr + 2, :])
                    eng.tensor_sub(v[:, 2, :], xt[:, lr + 2, :], xt[:, lr + 1, :])
                    eng.tensor_sub(v[:, 3, :], xt[:, lr + 1, :], xt[:, lr + 3, :])
                return va, vb

            vs = make_v(0)
            for ty in range(NTY):
                # keep two blocks of x prefetch in flight
                if blk_of[ty] >= 1 and ty == ty0s[blk_of[ty]]:
                    nxt = blk_of[ty] + 2
                    if nxt < len(TBLOCKS):
                        load_block(nxt)
                drains = []
                for bo in range(2):
                    accs = []
                    for i in range(4):
                        acc = psum.tile([128, W], f32, tag="acc")
                        for bi in range(2):
                            for kw in range(3):
                                nc.tensor.matmul(
                                    acc[:],
                                    wtap(bo, i, kw, bi),
                                    vs[bi][:, i, kw:kw + W],
                                    start=(bi == 0 and kw == 0),
                                    stop=(bi == 1 and kw == 2),
                                )
                        accs.append(acc)
                    drains.append(accs)
                # input transform for ty+1 goes ahead of this row's inverse
                # in the Vector queue so the PE never waits on it
                nvs = make_v(ty + 1) if ty + 1 < NTY else None
                for bo in range(2):
                    accs = drains[bo]
                    bvec = bias_s[:, bo:bo + 1]
                    ot = opool.tile([128, 2, W], f32, tag="out")
                    # Y0 = (M0 + b) + M1 + M2 ; Y1 = (-M3 + b) + M1 - M2
                    s0 = tpool.tile([128, W], f32, tag="s")
                    nc.scalar.activation(
                        s0[:], accs[0][:],
                        mybir.ActivationFunctionType.Identity, bias=bvec)
                    s1 = tpool.tile([128, W], f32, tag="s")
                    nc.scalar.activation(
                        s1[:], accs[3][:],
                        mybir.ActivationFunctionType.Identity, bias=bvec,
                        scale=-1.0)
                    t0 = tpool.tile([128, W], f32, tag="t")
                    nc.vector.tensor_add(t0[:], s0[:], accs[1][:])
                    nc.vector.tensor_add(ot[:, 0, :], t0[:], accs[2][:])
                    t1 = tpool.tile([128, W], f32, tag="t")
                    nc.vector.tensor_add(t1[:], s1[:], accs[1][:])
                    nc.vector.tensor_sub(ot[:, 1, :], t1[:], accs[2][:])
                    nc.sync.dma_start(out_d[:, bo, 2 * ty:2 * ty + 2, :],
                                      ot[:])
                vs = nvs

    nc.compile()
    return nc


def _get_nc():
    if "nc" not in _CACHED_NC:
        _CACHED_NC["nc"] = _build_nc()
    return _CACHED_NC["nc"]


def _prep_inputs(x, W_, b):
    import ml_dtypes
    bf = ml_dtypes.bfloat16
    xs_all = np.zeros((NCORES, CIN, RPC + 2, WPAD), bf)
    xb = x.astype(bf)
    for m in range(NCORES):
        g0 = max(0, m * RPC - 1)
        g1 = min(H, m * RPC + RPC + 1)
        r0 = g0 - (m * RPC - 1)
        xs_all[m, :, r0:r0 + (g1 - g0), 1:1 + W] = xb[:, g0:g1, :]
    # winograd F(2,3) weight transform along kh
    G = np.array([[1, 0, 0], [0.5, 0.5, 0.5], [0.5, -0.5, 0.5], [0, 0, 1]],
                 np.float32)
    U = np.einsum('ik,kwab->iwab', G, W_)      # [4, 3, CIN, COUT]
    # [i, kw, ci, co] -> [ci_lo, co_hi, i, kw, ci_hi, co_lo] -> [128, 48, 128]
    wt = np.ascontiguousarray(
        U.reshape(4, 3, 2, 128, 2, 128).transpose(3, 4, 0, 1, 2, 5)
        .reshape(128, NTAPS, 128).astype(bf))
    bias = np.ascontiguousarray(b.reshape(2, 128).T)
    return xs_all, wt, bias


def kernel(x, W, b, _trace=False):
    x = np.asarray(x, dtype=np.float32)
    W = np.asarray(W, dtype=np.float32)
    b = np.asarray(b, dtype=np.float32)
    nc = _get_nc()
    xs_all, wt, bias = _prep_inputs(x, W, b)
    in_maps = [{"xs": xs_all[m], "wt": wt, "bias": bias} for m in range(NCORES)]
    res = bass_utils.run_bass_kernel_spmd(
        nc, in_maps, list(range(NCORES)), trace=_trace)
    arr = np.stack([res.results[m]["out"] for m in range(NCORES)], axis=0)
    # [m, p, bo, yy, x] -> [bo, p, m, yy, x] -> [C_out, H, W]
    full = arr.transpose(2, 1, 0, 3, 4).reshape(COUT, H, 512)
    if _trace:
        return full, res
    return full
